# revision 28
# baseline (speedup 1.0000x reference)
"""Bahdanau attention kernel for Trainium2 (Bass/Tile), 8 NeuronCores.

Problem (per batch element b):
    q_proj = query[b] @ w1.T          # (LQ, H)
    k_proj = key[b]   @ w2.T          # (LK, H)
    score[q, k] = sum_h v[h] * tanh(q_proj[q, h] + k_proj[k, h])
    attn = softmax(score, axis=-1)    # output 1
    ctx  = attn @ value[b]            # output 2

Sharding: data-parallel over batch B=8 across the 8 cores (no collectives).
Host prep: query/key/w1/w2/v are passed pre-transposed so the device never
transposes inputs (contraction dim must live on SBUF partitions for the PE).

Per-core device schedule:
  - qpT/kpT = [h=128, l=512] projections via float32r matmuls.
  - Main loop over 64 subtiles of 8 queries:
      DVE tensor_scalar_add broadcasts qpT[:, q] over kpT  -> s[h, 8, 512]
      ACT tanh over the whole [128, 4096] tile             -> t
      8 PE matmuls, vT [h,1] stationary, t[:, j, :] moving -> score rows (PSUM)
  - Per 128-query block: row max (negated) -> exp(bias=-max, accum_out=sums)
    -> reciprocal -> scale -> attn out; PE-transpose of p -> 4 matmuls with
    value -> scale -> ctx out.
"""

import numpy as np

import concourse.bass as bass
import concourse.mybir as mybir
import concourse.tile as tile
from concourse import bacc
from concourse.bass_utils import run_bass_kernel_spmd
from concourse.masks import make_identity

F32 = mybir.dt.float32
F32R = mybir.dt.float32r
BF16 = mybir.dt.bfloat16

B = 8
L = 512          # LQ == LK
D = 512          # DQ == DK == DV
H = 128
P = 128          # SBUF partitions
NDB = D // P     # 4 d-blocks
NQB = L // P     # 4 query blocks
QSUB = 16        # queries per tanh subtile
NSUB = P // QSUB # subtiles per query block

_CACHED_NC = None


def _build_nc():
    nc = bacc.Bacc("TRN2", target_bir_lowering=False, debug=False)

    qT = nc.dram_tensor("qT", [D, L], BF16, kind="ExternalInput")     # query[b].T
    kT = nc.dram_tensor("kT", [D, L], BF16, kind="ExternalInput")     # key[b].T
    val = nc.dram_tensor("val", [L, D], BF16, kind="ExternalInput")   # value[b]
    w1T = nc.dram_tensor("w1T", [D, H], BF16, kind="ExternalInput")   # w1.T
    w2T = nc.dram_tensor("w2T", [D, H], BF16, kind="ExternalInput")   # w2.T
    # vwin[h, c] = v[h] if c == P else 0.  lhsT = vwin[:, P-r : 2P-r] puts v
    # in stationary column r, so matmul r adds score row r into the PSUM tile
    # (and +0 into every other row).
    vwin = nc.dram_tensor("vwin", [H, 2 * P], BF16, kind="ExternalInput")
    attn = nc.dram_tensor("attn", [L, L], F32, kind="ExternalOutput")
    ctxo = nc.dram_tensor("ctx", [L, L], F32, kind="ExternalOutput")

    with tile.TileContext(nc) as tc:
        with (
            tc.tile_pool(name="const", bufs=1) as const,
            tc.tile_pool(name="s", bufs=3) as s_pool,
            tc.tile_pool(name="t", bufs=3) as t_pool,
            tc.tile_pool(name="p", bufs=2) as p_pool,
            tc.tile_pool(name="pt", bufs=8) as pt_pool,
            tc.tile_pool(name="outs", bufs=4) as out_pool,
            tc.tile_pool(name="stat", bufs=12) as stat_pool,
            tc.tile_pool(name="proj_ps", bufs=1, space="PSUM") as proj_ps_pool,
            tc.tile_pool(name="score_ps", bufs=2, space="PSUM") as score_ps_pool,
            tc.tile_pool(name="ctx_ps", bufs=2, space="PSUM") as ctx_ps_pool,
            tc.tile_pool(name="tp_ps", bufs=2, space="PSUM") as tp_ps_pool,
        ):
            # ---------------- prologue: loads ----------------
            ident = const.tile([P, P], F32)
            make_identity(nc, ident[:])

            # Prologue loads split across the two HWDGE queues (Sync and
            # Scalar — the ACT engine is idle until the projections finish).
            vwin_sb = const.tile([H, 2 * P], BF16)
            nc.scalar.dma_start(out=vwin_sb[:], in_=vwin[:, :])

            qT_sb = const.tile([P, NDB, L], BF16)
            kT_sb = const.tile([P, NDB, L], BF16)
            w1T_sb = const.tile([P, NDB, H], BF16)
            w2T_sb = const.tile([P, NDB, H], BF16)
            for db in range(NDB):
                for eng, sb, dram in (
                    (nc.sync, w1T_sb, w1T),
                    (nc.scalar, w2T_sb, w2T),
                    (nc.sync, qT_sb, qT),
                    (nc.scalar, kT_sb, kT),
                ):
                    eng.dma_start(
                        out=sb[:, db, :], in_=dram[db * P : (db + 1) * P, :]
                    )

            # value is not needed until the first context matmul (~60us in);
            # keep it off the prologue critical path (gpsimd SWDGE queue).
            val_sb = const.tile([P, NQB, D], BF16)
            nc.gpsimd.dma_start(
                out=val_sb[:], in_=val[:, :].rearrange("(kb p) d -> p kb d", p=P)
            )

            # PE pre-warm: HAM clock-gates the PE to 1.2 GHz until it has been
            # busy ~3.4us.  Burn idle PE time during the input DMAs so the
            # projections and first score matmuls run at 2.4 GHz.  vwin is the
            # first DMA to land, so warm on it (results are never read).
            warm_ps = proj_ps_pool.tile([P, L], F32, tag="warm")
            for _ in range(24):
                nc.tensor.matmul(
                    warm_ps[:, 0:P], vwin_sb[:, 0:P], vwin_sb[:, P : 2 * P]
                )

            # ---------------- projections: qpT/kpT [h, l] ----------------
            # qpT stays fp32: it feeds DVE tensor_scalar as the per-partition
            # scalar operand, which must be fp32.  kpT goes bf16 so the adds
            # hit the DVE 4x perf mode.
            qpT = const.tile([H, L], F32)
            kpT = const.tile([H, L], BF16)
            for dst, wt, xt in ((qpT, w1T_sb, qT_sb), (kpT, w2T_sb, kT_sb)):
                ps = proj_ps_pool.tile([H, L], F32)
                for db in range(NDB):
                    nc.tensor.matmul(
                        ps[:],
                        wt[:, db, :],
                        xt[:, db, :],
                        start=(db == 0),
                        stop=(db == NDB - 1),
                    )
                nc.vector.tensor_copy(dst[:], ps[:])

            # ---------------- main loop ----------------
            def make_softmax_ctx(qb, score_ps):
                # Emitted 2 subtiles into the NEXT query block so the in-order
                # ACT engine never stalls on exp() waiting for this block's
                # final score matmuls.
                def emit():
                    neg_max = stat_pool.tile([P, 1], F32)
                    nc.vector.reduce_max(
                        neg_max[:], score_ps[:], axis=mybir.AxisListType.X,
                        negate=True,
                    )
                    p_t = p_pool.tile([P, L], F32)
                    sums = stat_pool.tile([P, 1], F32)
                    nc.scalar.activation(
                        p_t[:],
                        score_ps[:],
                        mybir.ActivationFunctionType.Exp,
                        bias=neg_max[:],
                        accum_out=sums[:],
                    )
                    inv = stat_pool.tile([P, 1], F32)
                    nc.vector.reciprocal(inv[:], sums[:])

                    attn_t = out_pool.tile([P, L], F32)
                    nc.vector.tensor_scalar_mul(attn_t[:], p_t[:], inv[:])
                    nc.sync.dma_start(
                        out=attn[qb * P : (qb + 1) * P, :], in_=attn_t[:]
                    )

                    # context: ctx[qb] = (p @ value) * inv
                    pT_sbs = []
                    for kb in range(NQB):
                        tp = tp_ps_pool.tile([P, P], F32)
                        nc.tensor.transpose(
                            tp[:], p_t[:, kb * P : (kb + 1) * P], ident[:]
                        )
                        pT_sb = pt_pool.tile([P, P], BF16)
                        nc.vector.tensor_copy(pT_sb[:], tp[:])
                        pT_sbs.append(pT_sb)
                    ctx_ps = ctx_ps_pool.tile([P, D], F32)
                    for kb in range(NQB):
                        nc.tensor.matmul(
                            ctx_ps[:],
                            pT_sbs[kb][:],
                            val_sb[:, kb, :],
                            start=(kb == 0),
                            stop=(kb == NQB - 1),
                        )
                    ctx_t = out_pool.tile([P, D], F32)
                    nc.vector.tensor_scalar_mul(ctx_t[:], ctx_ps[:], inv[:])
                    nc.sync.dma_start(
                        out=ctxo[qb * P : (qb + 1) * P, :], in_=ctx_t[:]
                    )

                return emit

            pending = None
            for qb in range(NQB):
                score_ps = score_ps_pool.tile([P, L], F32)
                for sub in range(NSUB):
                    s_t = s_pool.tile([P, QSUB, L], BF16)
                    for j in range(QSUB):
                        q = qb * P + sub * QSUB + j
                        nc.vector.tensor_scalar_add(
                            s_t[:, j, :], kpT[:], qpT[:, q : q + 1]
                        )
                    t_t = t_pool.tile([P, QSUB, L], BF16)
                    nc.scalar.activation(
                        t_t[:], s_t[:], mybir.ActivationFunctionType.Tanh
                    )
                    for j in range(QSUB):
                        row = sub * QSUB + j
                        nc.tensor.matmul(
                            score_ps[:],
                            vwin_sb[:, P - row : 2 * P - row],
                            t_t[:, j, :],
                            start=(row == 0),
                            stop=(row == P - 1),
                        )
                    # HAM keepalive: the PE finishes this subtile's matmuls
                    # ~3.5us before the next tanh lands; pad with discarded
                    # matmuls (anchored on t_t) so the idle window stays under
                    # the ~3.4us re-throttle threshold.
                    for _ in range(4):
                        nc.tensor.matmul(
                            warm_ps[:],
                            vwin_sb[:, 0:P],
                            t_t[:, 0, :],
                        )
                    if pending is not None and sub == 1:
                        pending()
                        pending = None
                pending = make_softmax_ctx(qb, score_ps)
            pending()

    nc.compile()
    return nc


def _get_nc():
    global _CACHED_NC
    if _CACHED_NC is None:
        _CACHED_NC = _build_nc()
    return _CACHED_NC


def _in_maps(query, key, value, w1, w2, v):
    import ml_dtypes

    f = np.float32
    bf = ml_dtypes.bfloat16
    w1T = np.ascontiguousarray(np.asarray(w1, dtype=f).T.astype(bf))
    w2T = np.ascontiguousarray(np.asarray(w2, dtype=f).T.astype(bf))
    vwin = np.zeros((H, 2 * P), dtype=bf)
    vwin[:, P] = np.asarray(v, dtype=f)[0].astype(bf)
    maps = []
    for b in range(B):
        maps.append(
            {
                "qT": np.ascontiguousarray(np.asarray(query[b], dtype=f).T.astype(bf)),
                "kT": np.ascontiguousarray(np.asarray(key[b], dtype=f).T.astype(bf)),
                "val": np.ascontiguousarray(np.asarray(value[b], dtype=f).astype(bf)),
                "w1T": w1T,
                "w2T": w2T,
                "vwin": vwin,
            }
        )
    return maps


def run(query, key, value, w1, w2, v, trace=False, **spmd_kwargs):
    nc = _get_nc()
    res = run_bass_kernel_spmd(
        nc,
        _in_maps(query, key, value, w1, w2, v),
        list(range(B)),
        trace=trace,
        **spmd_kwargs,
    )
    attn = np.stack([res.results[b]["attn"] for b in range(B)])
    ctx = np.stack([res.results[b]["ctx"] for b in range(B)])
    return (attn, ctx), res


def kernel(query, key, value, w1, w2, v):
    (attn, ctx), _ = run(query, key, value, w1, w2, v, trace=False)
    return (attn, ctx)


# revision 35
# speedup vs baseline: 1.0330x; 1.0330x over previous
"""Bahdanau attention kernel for Trainium2 (Bass/Tile), 8 NeuronCores.

Problem (per batch element b):
    q_proj = query[b] @ w1.T          # (LQ, H)
    k_proj = key[b]   @ w2.T          # (LK, H)
    score[q, k] = sum_h v[h] * tanh(q_proj[q, h] + k_proj[k, h])
    attn = softmax(score, axis=-1)    # output 1
    ctx  = attn @ value[b]            # output 2

Sharding: data-parallel over batch B=8 across the 8 cores (no collectives).
Host prep: query/key/w1/w2/v are passed pre-transposed so the device never
transposes inputs (contraction dim must live on SBUF partitions for the PE).

Per-core device schedule:
  - qpT/kpT = [h=128, l=512] projections via float32r matmuls.
  - Main loop over 64 subtiles of 8 queries:
      DVE tensor_scalar_add broadcasts qpT[:, q] over kpT  -> s[h, 8, 512]
      ACT tanh over the whole [128, 4096] tile             -> t
      8 PE matmuls, vT [h,1] stationary, t[:, j, :] moving -> score rows (PSUM)
  - Per 128-query block: row max (negated) -> exp(bias=-max, accum_out=sums)
    -> reciprocal -> scale -> attn out; PE-transpose of p -> 4 matmuls with
    value -> scale -> ctx out.
"""

import numpy as np

import concourse.bass as bass
import concourse.mybir as mybir
import concourse.tile as tile
from concourse import bacc
from concourse.bass_utils import run_bass_kernel_spmd
from concourse.masks import make_identity

F32 = mybir.dt.float32
F32R = mybir.dt.float32r
BF16 = mybir.dt.bfloat16

B = 8
L = 512          # LQ == LK
D = 512          # DQ == DK == DV
H = 128
P = 128          # SBUF partitions
NDB = D // P     # 4 d-blocks
NQB = L // P     # 4 query blocks
QSUB = 16        # queries per tanh subtile
NSUB = P // QSUB # subtiles per query block

_CACHED_NC = None


def _build_nc():
    nc = bacc.Bacc("TRN2", target_bir_lowering=False, debug=False)

    # All inputs arrive pre-tiled on the host so each SBUF partition's data is
    # one contiguous DRAM line (fat DMA descriptors: 1-4KB per partition).
    qT = nc.dram_tensor("qT", [P, NDB, L], BF16, kind="ExternalInput")
    kT = nc.dram_tensor("kT", [P, NDB, L], BF16, kind="ExternalInput")
    val = nc.dram_tensor("val", [P, NQB, D], BF16, kind="ExternalInput")
    w1T = nc.dram_tensor("w1T", [P, NDB, H], BF16, kind="ExternalInput")
    w2T = nc.dram_tensor("w2T", [P, NDB, H], BF16, kind="ExternalInput")
    # vwin[h, c] = v[h] if c == P else 0.  lhsT = vwin[:, P-r : 2P-r] puts v
    # in stationary column r, so matmul r adds score row r into the PSUM tile
    # (and +0 into every other row).
    vwin = nc.dram_tensor("vwin", [H, 2 * P], BF16, kind="ExternalInput")
    attn = nc.dram_tensor("attn", [L, L], F32, kind="ExternalOutput")
    ctxo = nc.dram_tensor("ctx", [L, L], F32, kind="ExternalOutput")

    with tile.TileContext(nc) as tc:
        with (
            tc.tile_pool(name="const", bufs=1) as const,
            tc.tile_pool(name="s", bufs=3) as s_pool,
            tc.tile_pool(name="t", bufs=3) as t_pool,
            tc.tile_pool(name="p", bufs=2) as p_pool,
            tc.tile_pool(name="pt", bufs=8) as pt_pool,
            tc.tile_pool(name="outs", bufs=4) as out_pool,
            tc.tile_pool(name="stat", bufs=12) as stat_pool,
            tc.tile_pool(name="proj_ps", bufs=2, space="PSUM") as proj_ps_pool,
            tc.tile_pool(name="warm_ps", bufs=1, space="PSUM") as warm_ps_pool,
            tc.tile_pool(name="score_ps", bufs=2, space="PSUM") as score_ps_pool,
            tc.tile_pool(name="ctx_ps", bufs=2, space="PSUM") as ctx_ps_pool,
            tc.tile_pool(name="tp_ps", bufs=1, space="PSUM") as tp_ps_pool,
        ):
            # ---------------- prologue: loads ----------------
            ident = const.tile([P, P], F32)
            make_identity(nc, ident[:])

            # Prologue loads split across the two HWDGE queues (Sync and
            # Scalar — the ACT engine is idle until the projections finish).
            vwin_sb = const.tile([H, 2 * P], BF16)
            nc.scalar.dma_start(out=vwin_sb[:], in_=vwin[:, :])

            qT_sb = const.tile([P, NDB, L], BF16)
            kT_sb = const.tile([P, NDB, L], BF16)
            w1T_sb = const.tile([P, NDB, H], BF16)
            w2T_sb = const.tile([P, NDB, H], BF16)
            nc.sync.dma_start(out=w1T_sb[:], in_=w1T[:])
            nc.scalar.dma_start(out=w2T_sb[:], in_=w2T[:])
            nc.sync.dma_start(out=qT_sb[:], in_=qT[:])
            nc.scalar.dma_start(out=kT_sb[:], in_=kT[:])

            # value is not needed until the first context matmul (~60us in);
            # keep it off the prologue critical path (gpsimd SWDGE queue).
            val_sb = const.tile([P, NQB, D], BF16)
            nc.gpsimd.dma_start(out=val_sb[:], in_=val[:])

            # PE pre-warm: HAM clock-gates the PE to 1.2 GHz until it has been
            # busy ~3.4us.  Burn idle PE time during the input DMAs so the
            # projections and first score matmuls run at 2.4 GHz.  vwin is the
            # first DMA to land, so warm on it (results are never read).
            warm_ps = warm_ps_pool.tile([P, L], F32)
            for _ in range(24):
                nc.tensor.matmul(
                    warm_ps[:, 0:P], vwin_sb[:, 0:P], vwin_sb[:, P : 2 * P]
                )

            # ---------------- projections: qpT/kpT [h, l] ----------------
            # qpT stays fp32: it feeds DVE tensor_scalar as the per-partition
            # scalar operand, which must be fp32.  kpT goes bf16 so the adds
            # hit the DVE 4x perf mode.  The two projections interleave
            # db-wise so both finish as soon as their DMAs land.
            qpT = const.tile([H, L], F32)
            kpT = const.tile([H, L], BF16)
            ps_q = proj_ps_pool.tile([H, L], F32, tag="proj")
            ps_k = proj_ps_pool.tile([H, L], F32, tag="proj")
            for db in range(NDB):
                nc.tensor.matmul(
                    ps_q[:], w1T_sb[:, db, :], qT_sb[:, db, :],
                    start=(db == 0), stop=(db == NDB - 1),
                )
                nc.tensor.matmul(
                    ps_k[:], w2T_sb[:, db, :], kT_sb[:, db, :],
                    start=(db == 0), stop=(db == NDB - 1),
                )
            nc.vector.tensor_copy(kpT[:], ps_k[:])
            nc.vector.tensor_copy(qpT[:], ps_q[:])

            # ---------------- main loop ----------------
            def make_softmax_ctx(qb, score_ps):
                # Emitted 2 subtiles into the NEXT query block so the in-order
                # ACT engine never stalls on exp() waiting for this block's
                # final score matmuls.
                def emit():
                    neg_max = stat_pool.tile([P, 1], F32)
                    nc.vector.reduce_max(
                        neg_max[:], score_ps[:], axis=mybir.AxisListType.X,
                        negate=True,
                    )
                    p_t = p_pool.tile([P, L], F32)
                    sums = stat_pool.tile([P, 1], F32)
                    nc.scalar.activation(
                        p_t[:],
                        score_ps[:],
                        mybir.ActivationFunctionType.Exp,
                        bias=neg_max[:],
                        accum_out=sums[:],
                    )
                    inv = stat_pool.tile([P, 1], F32)
                    nc.vector.reciprocal(inv[:], sums[:])

                    attn_t = out_pool.tile([P, L], F32)
                    nc.vector.tensor_scalar_mul(attn_t[:], p_t[:], inv[:])
                    nc.sync.dma_start(
                        out=attn[qb * P : (qb + 1) * P, :], in_=attn_t[:]
                    )

                    # context: ctx[qb] = (p @ value) * inv
                    pT_sbs = []
                    for kb in range(NQB):
                        tp = tp_ps_pool.tile([P, P], F32)
                        nc.tensor.transpose(
                            tp[:], p_t[:, kb * P : (kb + 1) * P], ident[:]
                        )
                        pT_sb = pt_pool.tile([P, P], BF16)
                        nc.vector.tensor_copy(pT_sb[:], tp[:])
                        pT_sbs.append(pT_sb)
                    ctx_ps = ctx_ps_pool.tile([P, D], F32)
                    for kb in range(NQB):
                        nc.tensor.matmul(
                            ctx_ps[:],
                            pT_sbs[kb][:],
                            val_sb[:, kb, :],
                            start=(kb == 0),
                            stop=(kb == NQB - 1),
                        )
                    ctx_t = out_pool.tile([P, D], F32)
                    nc.vector.tensor_scalar_mul(ctx_t[:], ctx_ps[:], inv[:])
                    nc.sync.dma_start(
                        out=ctxo[qb * P : (qb + 1) * P, :], in_=ctx_t[:]
                    )

                return emit

            pending = None
            for qb in range(NQB):
                score_ps = score_ps_pool.tile([P, L], F32)
                for sub in range(NSUB):
                    s_t = s_pool.tile([P, QSUB, L], BF16)
                    for j in range(QSUB):
                        q = qb * P + sub * QSUB + j
                        nc.vector.tensor_scalar_add(
                            s_t[:, j, :], kpT[:], qpT[:, q : q + 1]
                        )
                    t_t = t_pool.tile([P, QSUB, L], BF16)
                    first = qb == 0 and sub == 0
                    last = qb == NQB - 1 and sub == NSUB - 1
                    if first or last:
                        # Split the pipeline-fill / pipeline-drain tanh into
                        # halves so downstream (first) or upstream (last)
                        # work starts ~3.5us earlier.
                        half = QSUB // 2
                        nc.scalar.activation(
                            t_t[:, :half, :], s_t[:, :half, :],
                            mybir.ActivationFunctionType.Tanh,
                        )
                        nc.scalar.activation(
                            t_t[:, half:, :], s_t[:, half:, :],
                            mybir.ActivationFunctionType.Tanh,
                        )
                    else:
                        nc.scalar.activation(
                            t_t[:], s_t[:], mybir.ActivationFunctionType.Tanh
                        )
                    for j in range(QSUB):
                        row = sub * QSUB + j
                        nc.tensor.matmul(
                            score_ps[:],
                            vwin_sb[:, P - row : 2 * P - row],
                            t_t[:, j, :],
                            start=(row == 0),
                            stop=(row == P - 1),
                        )
                    # HAM keepalive: the PE finishes this subtile's matmuls
                    # ~3.5us before the next tanh lands; pad with discarded
                    # matmuls (anchored on t_t) so the idle window stays under
                    # the ~3.4us re-throttle threshold.
                    if not last:
                        for _ in range(4):
                            nc.tensor.matmul(
                                warm_ps[:],
                                vwin_sb[:, 0:P],
                                t_t[:, 0, :],
                            )
                    if pending is not None and sub == 1:
                        pending()
                        pending = None
                pending = make_softmax_ctx(qb, score_ps)
            pending()

    nc.compile()
    return nc


def _get_nc():
    global _CACHED_NC
    if _CACHED_NC is None:
        _CACHED_NC = _build_nc()
    return _CACHED_NC


def _in_maps(query, key, value, w1, w2, v):
    import ml_dtypes

    import ml_dtypes as _md

    f = np.float32
    bf = _md.bfloat16

    def tile_rows(arr):
        # [R, C] with R = NB*P  ->  [P, NB, C]: partition-major, so each
        # SBUF partition's data is one contiguous DRAM line.
        r, c = arr.shape
        nb = r // P
        return np.ascontiguousarray(arr.reshape(nb, P, c).transpose(1, 0, 2))

    w1T = tile_rows(np.asarray(w1, dtype=f).T.astype(bf))
    w2T = tile_rows(np.asarray(w2, dtype=f).T.astype(bf))
    vwin = np.zeros((H, 2 * P), dtype=bf)
    vwin[:, P] = np.asarray(v, dtype=f)[0].astype(bf)
    maps = []
    for b in range(B):
        maps.append(
            {
                "qT": tile_rows(np.asarray(query[b], dtype=f).T.astype(bf)),
                "kT": tile_rows(np.asarray(key[b], dtype=f).T.astype(bf)),
                "val": tile_rows(np.asarray(value[b], dtype=f).astype(bf)),
                "w1T": w1T,
                "w2T": w2T,
                "vwin": vwin,
            }
        )
    return maps


def run(query, key, value, w1, w2, v, trace=False, **spmd_kwargs):
    nc = _get_nc()
    res = run_bass_kernel_spmd(
        nc,
        _in_maps(query, key, value, w1, w2, v),
        list(range(B)),
        trace=trace,
        **spmd_kwargs,
    )
    attn = np.stack([res.results[b]["attn"] for b in range(B)])
    ctx = np.stack([res.results[b]["ctx"] for b in range(B)])
    return (attn, ctx), res


def kernel(query, key, value, w1, w2, v):
    (attn, ctx), _ = run(query, key, value, w1, w2, v, trace=False)
    return (attn, ctx)


# revision 36
# speedup vs baseline: 1.0341x; 1.0011x over previous
"""Bahdanau attention kernel for Trainium2 (Bass/Tile), 8 NeuronCores.

Problem (per batch element b):
    q_proj = query[b] @ w1.T          # (LQ, H)
    k_proj = key[b]   @ w2.T          # (LK, H)
    score[q, k] = sum_h v[h] * tanh(q_proj[q, h] + k_proj[k, h])
    attn = softmax(score, axis=-1)    # output 1
    ctx  = attn @ value[b]            # output 2

Sharding: data-parallel over batch B=8 across the 8 cores (no collectives).
Host prep: query/key/w1/w2/v are passed pre-transposed so the device never
transposes inputs (contraction dim must live on SBUF partitions for the PE).

Per-core device schedule:
  - qpT/kpT = [h=128, l=512] projections via float32r matmuls.
  - Main loop over 64 subtiles of 8 queries:
      DVE tensor_scalar_add broadcasts qpT[:, q] over kpT  -> s[h, 8, 512]
      ACT tanh over the whole [128, 4096] tile             -> t
      8 PE matmuls, vT [h,1] stationary, t[:, j, :] moving -> score rows (PSUM)
  - Per 128-query block: row max (negated) -> exp(bias=-max, accum_out=sums)
    -> reciprocal -> scale -> attn out; PE-transpose of p -> 4 matmuls with
    value -> scale -> ctx out.
"""

import numpy as np

import concourse.bass as bass
import concourse.mybir as mybir
import concourse.tile as tile
from concourse import bacc
from concourse.bass_utils import run_bass_kernel_spmd
from concourse.masks import make_identity

F32 = mybir.dt.float32
F32R = mybir.dt.float32r
BF16 = mybir.dt.bfloat16

B = 8
L = 512          # LQ == LK
D = 512          # DQ == DK == DV
H = 128
P = 128          # SBUF partitions
NDB = D // P     # 4 d-blocks
NQB = L // P     # 4 query blocks
QSUB = 16        # queries per tanh subtile
NSUB = P // QSUB # subtiles per query block

_CACHED_NC = None


def _build_nc():
    nc = bacc.Bacc("TRN2", target_bir_lowering=False, debug=False)

    # All inputs arrive pre-tiled on the host so each SBUF partition's data is
    # one contiguous DRAM line (fat DMA descriptors: 1-4KB per partition).
    qT = nc.dram_tensor("qT", [P, NDB, L], BF16, kind="ExternalInput")
    kT = nc.dram_tensor("kT", [P, NDB, L], BF16, kind="ExternalInput")
    val = nc.dram_tensor("val", [P, NQB, D], BF16, kind="ExternalInput")
    w1T = nc.dram_tensor("w1T", [P, NDB, H], BF16, kind="ExternalInput")
    w2T = nc.dram_tensor("w2T", [P, NDB, H], BF16, kind="ExternalInput")
    # vwin[h, c] = v[h] if c == P else 0.  lhsT = vwin[:, P-r : 2P-r] puts v
    # in stationary column r, so matmul r adds score row r into the PSUM tile
    # (and +0 into every other row).
    vwin = nc.dram_tensor("vwin", [H, 2 * P], BF16, kind="ExternalInput")
    attn = nc.dram_tensor("attn", [L, L], F32, kind="ExternalOutput")
    ctxo = nc.dram_tensor("ctx", [L, L], F32, kind="ExternalOutput")

    with tile.TileContext(nc) as tc:
        with (
            tc.tile_pool(name="const", bufs=1) as const,
            tc.tile_pool(name="s", bufs=3) as s_pool,
            tc.tile_pool(name="t", bufs=3) as t_pool,
            tc.tile_pool(name="p", bufs=2) as p_pool,
            tc.tile_pool(name="pt", bufs=8) as pt_pool,
            tc.tile_pool(name="outs", bufs=4) as out_pool,
            tc.tile_pool(name="stat", bufs=12) as stat_pool,
            tc.tile_pool(name="proj_ps", bufs=2, space="PSUM") as proj_ps_pool,
            tc.tile_pool(name="warm_ps", bufs=1, space="PSUM") as warm_ps_pool,
            tc.tile_pool(name="score_ps", bufs=2, space="PSUM") as score_ps_pool,
            tc.tile_pool(name="ctx_ps", bufs=1, space="PSUM") as ctx_ps_pool,
            tc.tile_pool(name="tp_ps", bufs=2, space="PSUM") as tp_ps_pool,
        ):
            # ---------------- prologue: loads ----------------
            ident = const.tile([P, P], F32)
            make_identity(nc, ident[:])

            # Prologue loads split across the two HWDGE queues (Sync and
            # Scalar — the ACT engine is idle until the projections finish).
            vwin_sb = const.tile([H, 2 * P], BF16)
            nc.scalar.dma_start(out=vwin_sb[:], in_=vwin[:, :])

            qT_sb = const.tile([P, NDB, L], BF16)
            kT_sb = const.tile([P, NDB, L], BF16)
            w1T_sb = const.tile([P, NDB, H], BF16)
            w2T_sb = const.tile([P, NDB, H], BF16)
            nc.sync.dma_start(out=w1T_sb[:], in_=w1T[:])
            nc.scalar.dma_start(out=w2T_sb[:], in_=w2T[:])
            nc.sync.dma_start(out=qT_sb[:], in_=qT[:])
            nc.scalar.dma_start(out=kT_sb[:], in_=kT[:])

            # value is not needed until the first context matmul (~60us in);
            # keep it off the prologue critical path (gpsimd SWDGE queue).
            val_sb = const.tile([P, NQB, D], BF16)
            nc.gpsimd.dma_start(out=val_sb[:], in_=val[:])

            # PE pre-warm: HAM clock-gates the PE to 1.2 GHz until it has been
            # busy ~3.4us.  Burn idle PE time during the input DMAs so the
            # projections and first score matmuls run at 2.4 GHz.  vwin is the
            # first DMA to land, so warm on it (results are never read).
            warm_ps = warm_ps_pool.tile([P, L], F32)
            for _ in range(24):
                nc.tensor.matmul(
                    warm_ps[:, 0:P], vwin_sb[:, 0:P], vwin_sb[:, P : 2 * P]
                )

            # ---------------- projections: qpT/kpT [h, l] ----------------
            # qpT stays fp32: it feeds DVE tensor_scalar as the per-partition
            # scalar operand, which must be fp32.  kpT goes bf16 so the adds
            # hit the DVE 4x perf mode.  The two projections interleave
            # db-wise so both finish as soon as their DMAs land.
            qpT = const.tile([H, L], F32)
            kpT = const.tile([H, L], BF16)
            ps_q = proj_ps_pool.tile([H, L], F32, tag="proj")
            ps_k = proj_ps_pool.tile([H, L], F32, tag="proj")
            for db in range(NDB):
                nc.tensor.matmul(
                    ps_q[:], w1T_sb[:, db, :], qT_sb[:, db, :],
                    start=(db == 0), stop=(db == NDB - 1),
                )
                nc.tensor.matmul(
                    ps_k[:], w2T_sb[:, db, :], kT_sb[:, db, :],
                    start=(db == 0), stop=(db == NDB - 1),
                )
            nc.vector.tensor_copy(kpT[:], ps_k[:])
            nc.vector.tensor_copy(qpT[:], ps_q[:])

            # ---------------- main loop ----------------
            def make_softmax_ctx(qb, score_ps):
                # Emitted 2 subtiles into the NEXT query block so the in-order
                # ACT engine never stalls on exp() waiting for this block's
                # final score matmuls.
                def emit():
                    neg_max = stat_pool.tile([P, 1], F32)
                    nc.vector.reduce_max(
                        neg_max[:], score_ps[:], axis=mybir.AxisListType.X,
                        negate=True,
                    )
                    p_t = p_pool.tile([P, L], F32)
                    sums = stat_pool.tile([P, 1], F32)
                    nc.scalar.activation(
                        p_t[:],
                        score_ps[:],
                        mybir.ActivationFunctionType.Exp,
                        bias=neg_max[:],
                        accum_out=sums[:],
                    )
                    inv = stat_pool.tile([P, 1], F32)
                    nc.vector.reciprocal(inv[:], sums[:])

                    attn_t = out_pool.tile([P, L], F32)
                    nc.vector.tensor_scalar_mul(attn_t[:], p_t[:], inv[:])
                    nc.sync.dma_start(
                        out=attn[qb * P : (qb + 1) * P, :], in_=attn_t[:]
                    )

                    # context: ctx[qb] = (p @ value) * inv
                    pT_sbs = []
                    for kb in range(NQB):
                        tp = tp_ps_pool.tile([P, P], F32)
                        nc.tensor.transpose(
                            tp[:], p_t[:, kb * P : (kb + 1) * P], ident[:]
                        )
                        pT_sb = pt_pool.tile([P, P], BF16)
                        nc.vector.tensor_copy(pT_sb[:], tp[:])
                        pT_sbs.append(pT_sb)
                    ctx_ps = ctx_ps_pool.tile([P, D], F32)
                    for kb in range(NQB):
                        nc.tensor.matmul(
                            ctx_ps[:],
                            pT_sbs[kb][:],
                            val_sb[:, kb, :],
                            start=(kb == 0),
                            stop=(kb == NQB - 1),
                        )
                    ctx_t = out_pool.tile([P, D], F32)
                    nc.vector.tensor_scalar_mul(ctx_t[:], ctx_ps[:], inv[:])
                    nc.sync.dma_start(
                        out=ctxo[qb * P : (qb + 1) * P, :], in_=ctx_t[:]
                    )

                return emit

            pending = None
            for qb in range(NQB):
                score_ps = score_ps_pool.tile([P, L], F32)
                for sub in range(NSUB):
                    s_t = s_pool.tile([P, QSUB, L], BF16)
                    for j in range(QSUB):
                        q = qb * P + sub * QSUB + j
                        nc.vector.tensor_scalar_add(
                            s_t[:, j, :], kpT[:], qpT[:, q : q + 1]
                        )
                    t_t = t_pool.tile([P, QSUB, L], BF16)
                    first = qb == 0 and sub == 0
                    last = qb == NQB - 1 and sub == NSUB - 1
                    if first or last:
                        # Split the pipeline-fill / pipeline-drain tanh into
                        # halves so downstream (first) or upstream (last)
                        # work starts ~3.5us earlier.
                        half = QSUB // 2
                        nc.scalar.activation(
                            t_t[:, :half, :], s_t[:, :half, :],
                            mybir.ActivationFunctionType.Tanh,
                        )
                        nc.scalar.activation(
                            t_t[:, half:, :], s_t[:, half:, :],
                            mybir.ActivationFunctionType.Tanh,
                        )
                    else:
                        nc.scalar.activation(
                            t_t[:], s_t[:], mybir.ActivationFunctionType.Tanh
                        )
                    for j in range(QSUB):
                        row = sub * QSUB + j
                        nc.tensor.matmul(
                            score_ps[:],
                            vwin_sb[:, P - row : 2 * P - row],
                            t_t[:, j, :],
                            start=(row == 0),
                            stop=(row == P - 1),
                        )
                    # HAM keepalive: the PE finishes this subtile's matmuls
                    # ~3.5us before the next tanh lands; pad with discarded
                    # matmuls (anchored on t_t) so the idle window stays under
                    # the ~3.4us re-throttle threshold.
                    if not last:
                        for _ in range(4):
                            nc.tensor.matmul(
                                warm_ps[:],
                                vwin_sb[:, 0:P],
                                t_t[:, 0, :],
                            )
                    if pending is not None and sub == 1:
                        pending()
                        pending = None
                pending = make_softmax_ctx(qb, score_ps)
            pending()

    nc.compile()
    return nc


def _get_nc():
    global _CACHED_NC
    if _CACHED_NC is None:
        _CACHED_NC = _build_nc()
    return _CACHED_NC


def _in_maps(query, key, value, w1, w2, v):
    import ml_dtypes

    import ml_dtypes as _md

    f = np.float32
    bf = _md.bfloat16

    def tile_rows(arr):
        # [R, C] with R = NB*P  ->  [P, NB, C]: partition-major, so each
        # SBUF partition's data is one contiguous DRAM line.
        r, c = arr.shape
        nb = r // P
        return np.ascontiguousarray(arr.reshape(nb, P, c).transpose(1, 0, 2))

    w1T = tile_rows(np.asarray(w1, dtype=f).T.astype(bf))
    w2T = tile_rows(np.asarray(w2, dtype=f).T.astype(bf))
    vwin = np.zeros((H, 2 * P), dtype=bf)
    vwin[:, P] = np.asarray(v, dtype=f)[0].astype(bf)
    maps = []
    for b in range(B):
        maps.append(
            {
                "qT": tile_rows(np.asarray(query[b], dtype=f).T.astype(bf)),
                "kT": tile_rows(np.asarray(key[b], dtype=f).T.astype(bf)),
                "val": tile_rows(np.asarray(value[b], dtype=f).astype(bf)),
                "w1T": w1T,
                "w2T": w2T,
                "vwin": vwin,
            }
        )
    return maps


def run(query, key, value, w1, w2, v, trace=False, **spmd_kwargs):
    nc = _get_nc()
    res = run_bass_kernel_spmd(
        nc,
        _in_maps(query, key, value, w1, w2, v),
        list(range(B)),
        trace=trace,
        **spmd_kwargs,
    )
    attn = np.stack([res.results[b]["attn"] for b in range(B)])
    ctx = np.stack([res.results[b]["ctx"] for b in range(B)])
    return (attn, ctx), res


def kernel(query, key, value, w1, w2, v):
    (attn, ctx), _ = run(query, key, value, w1, w2, v, trace=False)
    return (attn, ctx)


# revision 38
# speedup vs baseline: 1.0363x; 1.0021x over previous
"""Bahdanau attention kernel for Trainium2 (Bass/Tile), 8 NeuronCores.

Problem (per batch element b):
    q_proj = query[b] @ w1.T          # (LQ, H)
    k_proj = key[b]   @ w2.T          # (LK, H)
    score[q, k] = sum_h v[h] * tanh(q_proj[q, h] + k_proj[k, h])
    attn = softmax(score, axis=-1)    # output 1
    ctx  = attn @ value[b]            # output 2

Sharding: data-parallel over batch B=8 across the 8 cores (no collectives).
Host prep: query/key/w1/w2/v are passed pre-transposed so the device never
transposes inputs (contraction dim must live on SBUF partitions for the PE).

Per-core device schedule:
  - qpT/kpT = [h=128, l=512] projections via float32r matmuls.
  - Main loop over 64 subtiles of 8 queries:
      DVE tensor_scalar_add broadcasts qpT[:, q] over kpT  -> s[h, 8, 512]
      ACT tanh over the whole [128, 4096] tile             -> t
      8 PE matmuls, vT [h,1] stationary, t[:, j, :] moving -> score rows (PSUM)
  - Per 128-query block: row max (negated) -> exp(bias=-max, accum_out=sums)
    -> reciprocal -> scale -> attn out; PE-transpose of p -> 4 matmuls with
    value -> scale -> ctx out.
"""

import numpy as np

import concourse.bass as bass
import concourse.mybir as mybir
import concourse.tile as tile
from concourse import bacc
from concourse.bass_utils import run_bass_kernel_spmd
from concourse.masks import make_identity

F32 = mybir.dt.float32
F32R = mybir.dt.float32r
BF16 = mybir.dt.bfloat16

B = 8
L = 512          # LQ == LK
D = 512          # DQ == DK == DV
H = 128
P = 128          # SBUF partitions
NDB = D // P     # 4 d-blocks
NQB = L // P     # 4 query blocks
QSUB = 16        # queries per tanh subtile
NSUB = P // QSUB # subtiles per query block

_CACHED_NC = None


def _build_nc():
    nc = bacc.Bacc("TRN2", target_bir_lowering=False, debug=False)

    # All inputs arrive pre-tiled on the host so each SBUF partition's data is
    # one contiguous DRAM line (fat DMA descriptors: 1-4KB per partition).
    qT = nc.dram_tensor("qT", [P, NDB, L], BF16, kind="ExternalInput")
    kT = nc.dram_tensor("kT", [P, NDB, L], BF16, kind="ExternalInput")
    val = nc.dram_tensor("val", [P, NQB, D], BF16, kind="ExternalInput")
    w1T = nc.dram_tensor("w1T", [P, NDB, H], BF16, kind="ExternalInput")
    w2T = nc.dram_tensor("w2T", [P, NDB, H], BF16, kind="ExternalInput")
    # vwin[h, c] = v[h] if c == P else 0.  lhsT = vwin[:, P-r : 2P-r] puts v
    # in stationary column r, so matmul r adds score row r into the PSUM tile
    # (and +0 into every other row).
    vwin = nc.dram_tensor("vwin", [H, 2 * P], BF16, kind="ExternalInput")
    attn = nc.dram_tensor("attn", [L, L], F32, kind="ExternalOutput")
    ctxo = nc.dram_tensor("ctx", [L, L], F32, kind="ExternalOutput")

    with tile.TileContext(nc) as tc:
        with (
            tc.tile_pool(name="const", bufs=1) as const,
            tc.tile_pool(name="s", bufs=3) as s_pool,
            tc.tile_pool(name="t", bufs=3) as t_pool,
            tc.tile_pool(name="p", bufs=2) as p_pool,
            tc.tile_pool(name="pt", bufs=8) as pt_pool,
            tc.tile_pool(name="outs", bufs=4) as out_pool,
            tc.tile_pool(name="stat", bufs=12) as stat_pool,
            tc.tile_pool(name="proj_ps", bufs=2, space="PSUM") as proj_ps_pool,
            tc.tile_pool(name="warm_ps", bufs=1, space="PSUM") as warm_ps_pool,
            tc.tile_pool(name="score_ps", bufs=2, space="PSUM") as score_ps_pool,
            tc.tile_pool(name="ctx_ps", bufs=1, space="PSUM") as ctx_ps_pool,
            tc.tile_pool(name="tp_ps", bufs=2, space="PSUM") as tp_ps_pool,
        ):
            # ---------------- prologue: loads ----------------
            ident = const.tile([P, P], F32)
            make_identity(nc, ident[:])

            # Prologue loads split across the two HWDGE queues (Sync and
            # Scalar — the ACT engine is idle until the projections finish).
            vwin_sb = const.tile([H, 2 * P], BF16)
            nc.scalar.dma_start(out=vwin_sb[:], in_=vwin[:, :])

            qT_sb = const.tile([P, NDB, L], BF16)
            kT_sb = const.tile([P, NDB, L], BF16)
            w1T_sb = const.tile([P, NDB, H], BF16)
            w2T_sb = const.tile([P, NDB, H], BF16)
            nc.sync.dma_start(out=w1T_sb[:], in_=w1T[:])
            nc.scalar.dma_start(out=w2T_sb[:], in_=w2T[:])
            hb = NDB // 2
            nc.sync.dma_start(out=qT_sb[:, :hb, :], in_=qT[:, :hb, :])
            nc.scalar.dma_start(out=kT_sb[:, :hb, :], in_=kT[:, :hb, :])
            nc.sync.dma_start(out=qT_sb[:, hb:, :], in_=qT[:, hb:, :])
            nc.scalar.dma_start(out=kT_sb[:, hb:, :], in_=kT[:, hb:, :])

            # value is not needed until the first context matmul (~60us in);
            # keep it off the prologue critical path (gpsimd SWDGE queue).
            val_sb = const.tile([P, NQB, D], BF16)
            nc.gpsimd.dma_start(out=val_sb[:], in_=val[:])

            # PE pre-warm: HAM clock-gates the PE to 1.2 GHz until it has been
            # busy ~3.4us.  Burn idle PE time during the input DMAs so the
            # projections and first score matmuls run at 2.4 GHz.  vwin is the
            # first DMA to land, so warm on it (results are never read).
            warm_ps = warm_ps_pool.tile([P, L], F32)
            for _ in range(10):
                nc.tensor.matmul(
                    warm_ps[:, 0:P], vwin_sb[:, 0:P], vwin_sb[:, P : 2 * P]
                )

            # ---------------- projections: qpT/kpT [h, l] ----------------
            # qpT stays fp32: it feeds DVE tensor_scalar as the per-partition
            # scalar operand, which must be fp32.  kpT goes bf16 so the adds
            # hit the DVE 4x perf mode.  The two projections interleave
            # db-wise so both finish as soon as their DMAs land.
            qpT = const.tile([H, L], F32)
            kpT = const.tile([H, L], BF16)
            ps_q = proj_ps_pool.tile([H, L], F32, tag="proj")
            ps_k = proj_ps_pool.tile([H, L], F32, tag="proj")
            for db in range(NDB):
                nc.tensor.matmul(
                    ps_q[:], w1T_sb[:, db, :], qT_sb[:, db, :],
                    start=(db == 0), stop=(db == NDB - 1),
                )
                nc.tensor.matmul(
                    ps_k[:], w2T_sb[:, db, :], kT_sb[:, db, :],
                    start=(db == 0), stop=(db == NDB - 1),
                )
            nc.vector.tensor_copy(kpT[:], ps_k[:])
            nc.vector.tensor_copy(qpT[:], ps_q[:])

            # ---------------- main loop ----------------
            def make_softmax_ctx(qb, score_ps):
                # Emitted 2 subtiles into the NEXT query block so the in-order
                # ACT engine never stalls on exp() waiting for this block's
                # final score matmuls.
                def emit():
                    neg_max = stat_pool.tile([P, 1], F32)
                    nc.vector.reduce_max(
                        neg_max[:], score_ps[:], axis=mybir.AxisListType.X,
                        negate=True,
                    )
                    p_t = p_pool.tile([P, L], F32)
                    sums = stat_pool.tile([P, 1], F32)
                    nc.scalar.activation(
                        p_t[:],
                        score_ps[:],
                        mybir.ActivationFunctionType.Exp,
                        bias=neg_max[:],
                        accum_out=sums[:],
                    )
                    inv = stat_pool.tile([P, 1], F32)
                    nc.vector.reciprocal(inv[:], sums[:])

                    attn_t = out_pool.tile([P, L], F32)
                    nc.vector.tensor_scalar_mul(attn_t[:], p_t[:], inv[:])
                    nc.sync.dma_start(
                        out=attn[qb * P : (qb + 1) * P, :], in_=attn_t[:]
                    )

                    # context: ctx[qb] = (p @ value) * inv
                    pT_sbs = []
                    for kb in range(NQB):
                        tp = tp_ps_pool.tile([P, P], F32)
                        nc.tensor.transpose(
                            tp[:], p_t[:, kb * P : (kb + 1) * P], ident[:]
                        )
                        pT_sb = pt_pool.tile([P, P], BF16)
                        nc.vector.tensor_copy(pT_sb[:], tp[:])
                        pT_sbs.append(pT_sb)
                    ctx_ps = ctx_ps_pool.tile([P, D], F32)
                    for kb in range(NQB):
                        nc.tensor.matmul(
                            ctx_ps[:],
                            pT_sbs[kb][:],
                            val_sb[:, kb, :],
                            start=(kb == 0),
                            stop=(kb == NQB - 1),
                        )
                    ctx_t = out_pool.tile([P, D], F32)
                    nc.vector.tensor_scalar_mul(ctx_t[:], ctx_ps[:], inv[:])
                    nc.sync.dma_start(
                        out=ctxo[qb * P : (qb + 1) * P, :], in_=ctx_t[:]
                    )

                return emit

            pending = None
            for qb in range(NQB):
                score_ps = score_ps_pool.tile([P, L], F32)
                for sub in range(NSUB):
                    s_t = s_pool.tile([P, QSUB, L], BF16)
                    for j in range(QSUB):
                        q = qb * P + sub * QSUB + j
                        nc.vector.tensor_scalar_add(
                            s_t[:, j, :], kpT[:], qpT[:, q : q + 1]
                        )
                    t_t = t_pool.tile([P, QSUB, L], BF16)
                    first = qb == 0 and sub == 0
                    last = qb == NQB - 1 and sub == NSUB - 1
                    if first or last:
                        # Split the pipeline-fill / pipeline-drain tanh into
                        # halves so downstream (first) or upstream (last)
                        # work starts ~3.5us earlier.
                        half = QSUB // 2
                        nc.scalar.activation(
                            t_t[:, :half, :], s_t[:, :half, :],
                            mybir.ActivationFunctionType.Tanh,
                        )
                        nc.scalar.activation(
                            t_t[:, half:, :], s_t[:, half:, :],
                            mybir.ActivationFunctionType.Tanh,
                        )
                    else:
                        nc.scalar.activation(
                            t_t[:], s_t[:], mybir.ActivationFunctionType.Tanh
                        )
                    for j in range(QSUB):
                        row = sub * QSUB + j
                        nc.tensor.matmul(
                            score_ps[:],
                            vwin_sb[:, P - row : 2 * P - row],
                            t_t[:, j, :],
                            start=(row == 0),
                            stop=(row == P - 1),
                        )
                    # HAM keepalive: the PE finishes this subtile's matmuls
                    # ~3.5us before the next tanh lands; pad with discarded
                    # matmuls (anchored on t_t) so the idle window stays under
                    # the ~3.4us re-throttle threshold.
                    if not last:
                        for _ in range(4):
                            nc.tensor.matmul(
                                warm_ps[:],
                                vwin_sb[:, 0:P],
                                t_t[:, 0, :],
                            )
                    if pending is not None and sub == 1:
                        pending()
                        pending = None
                pending = make_softmax_ctx(qb, score_ps)
            pending()

    nc.compile()
    return nc


def _get_nc():
    global _CACHED_NC
    if _CACHED_NC is None:
        _CACHED_NC = _build_nc()
    return _CACHED_NC


def _in_maps(query, key, value, w1, w2, v):
    import ml_dtypes

    import ml_dtypes as _md

    f = np.float32
    bf = _md.bfloat16

    def tile_rows(arr):
        # [R, C] with R = NB*P  ->  [P, NB, C]: partition-major, so each
        # SBUF partition's data is one contiguous DRAM line.
        r, c = arr.shape
        nb = r // P
        return np.ascontiguousarray(arr.reshape(nb, P, c).transpose(1, 0, 2))

    w1T = tile_rows(np.asarray(w1, dtype=f).T.astype(bf))
    w2T = tile_rows(np.asarray(w2, dtype=f).T.astype(bf))
    vwin = np.zeros((H, 2 * P), dtype=bf)
    vwin[:, P] = np.asarray(v, dtype=f)[0].astype(bf)
    maps = []
    for b in range(B):
        maps.append(
            {
                "qT": tile_rows(np.asarray(query[b], dtype=f).T.astype(bf)),
                "kT": tile_rows(np.asarray(key[b], dtype=f).T.astype(bf)),
                "val": tile_rows(np.asarray(value[b], dtype=f).astype(bf)),
                "w1T": w1T,
                "w2T": w2T,
                "vwin": vwin,
            }
        )
    return maps


def run(query, key, value, w1, w2, v, trace=False, **spmd_kwargs):
    nc = _get_nc()
    res = run_bass_kernel_spmd(
        nc,
        _in_maps(query, key, value, w1, w2, v),
        list(range(B)),
        trace=trace,
        **spmd_kwargs,
    )
    attn = np.stack([res.results[b]["attn"] for b in range(B)])
    ctx = np.stack([res.results[b]["ctx"] for b in range(B)])
    return (attn, ctx), res


def kernel(query, key, value, w1, w2, v):
    (attn, ctx), _ = run(query, key, value, w1, w2, v, trace=False)
    return (attn, ctx)
